# revision 27
# baseline (speedup 1.0000x reference)
"""Trainium2 Bass kernel for nn_CACProjector (logits = x @ W^T, CAC distances).

Strategy: data-parallel over batch B across 8 NeuronCores. Each core gets a
(768, 2048) column-slice xT of x^T (host-side transpose so the contraction
dim D lands on SBUF partitions) and a replicated W^T (768, 1024). On-core:

  logits[b, c] = sum_d xT[d, b] * wT[d, c]      (PE, fp32 accumulate in PSUM)
  sq_norm[b]   = sum_c logits[b, c]^2           (ACT Square pass w/ accum_out,
                                                 reads fp32 PSUM)
  dist[b, c]   = sqrt(sq_norm[b] + alpha^2 - 2*alpha*logits[b, c])
                                                 (ACT Sqrt w/ scale+bias,
                                                 reads fp32 PSUM)

d2 = ||l||^2 - 2a*l_j + a^2 >= (l_j - a)^2 >= 0 mathematically, and with this
data d2 ~ 1100 >> 0, so the reference's maximum(d2, 0) clamp is a no-op.

The kernel is HBM-bandwidth bound, so I/O transport precision is the main
lever. IO_MODE picks it:
  "bf16": x/W shipped bf16, logits/dist stored bf16 (fp32 PSUM accumulate and
          fp32 distance math throughout) -> ~13 MB/core of HBM traffic.
  "f32r": x/W shipped fp32 (TF32-rounded for full-rate PE), outputs fp32
          -> ~26 MB/core, rel err ~3e-4.
"""

import sys

sys.path.insert(0, "/opt/trn_rl_repo")

from contextlib import ExitStack

import ml_dtypes
import numpy as np

import concourse.tile as tile
from concourse import bacc, mybir
from concourse.bass_utils import run_bass_kernel_spmd

N_CORES = 8
B, D, C = 16384, 768, 1024
BS = B // N_CORES          # 2048 rows of B per core
P = 128                    # partition dim
KT = D // P                # 6 contraction chunks
NBT = BS // P              # 16 output row-tiles per core
ALPHA = 10.0

F32 = mybir.dt.float32
F32R = mybir.dt.float32r
BF16 = mybir.dt.bfloat16

IO_MODE = "bf16"


def build(io_mode=IO_MODE):
    in_dt = BF16 if io_mode == "bf16" else F32R
    out_dt = BF16 if io_mode == "bf16" else F32

    nc = bacc.Bacc("TRN2", target_bir_lowering=False, debug=False)
    xT = nc.dram_tensor("xT", [D, BS], in_dt, kind="ExternalInput").ap()
    wT = nc.dram_tensor("wT", [D, C], in_dt, kind="ExternalInput").ap()
    logits = nc.dram_tensor("logits", [BS, C], out_dt, kind="ExternalOutput").ap()
    dist = nc.dram_tensor("dist", [BS, C], out_dt, kind="ExternalOutput").ap()

    HB = BS // 2  # x tiles DMA'd in two free-dim halves for a faster ramp-in

    with tile.TileContext(nc) as tc, ExitStack() as ctx:
        xpool = ctx.enter_context(tc.tile_pool(name="xT", bufs=1))
        wpool = ctx.enter_context(tc.tile_pool(name="wT", bufs=1))
        psum = ctx.enter_context(tc.tile_pool(name="psum", bufs=4, space="PSUM"))
        lpool = ctx.enter_context(tc.tile_pool(name="lg", bufs=3))
        dpool = ctx.enter_context(tc.tile_pool(name="dist", bufs=3))
        spool = ctx.enter_context(tc.tile_pool(name="sq", bufs=2))
        npool = ctx.enter_context(tc.tile_pool(name="norms", bufs=3))

        # Ramp-in: per-k (x, w) pieces arrive in the order the k-major phase
        # consumes them; xb (not needed until b-tile 8, by which time the
        # ramp is over) rides one big 1.5 MB DMA for peak bandwidth.
        x0q0 = xpool.tile([P, 512], in_dt, tag="x0q0")
        nc.sync.dma_start(x0q0[:], xT[0:P, 0:512])
        w0lo = wpool.tile([P, 512], in_dt, tag="w0lo")
        nc.sync.dma_start(w0lo[:], wT[0:P, 0:512])
        w0hi = wpool.tile([P, 512], in_dt, tag="w0hi")
        nc.sync.dma_start(w0hi[:], wT[0:P, 512:1024])
        x0q1 = xpool.tile([P, 512], in_dt, tag="x0q1")
        nc.sync.dma_start(x0q1[:], xT[0:P, 512:1024])

        xa = {}
        wt_lo, wt_hi = [w0lo], [w0hi]
        for k in range(1, KT):
            xka = xpool.tile([P, HB], in_dt, tag=f"xa{k}")
            nc.sync.dma_start(xka[:], xT[k * P : (k + 1) * P, :HB])
            xa[k] = xka
            wk = wpool.tile([P, C], in_dt, tag=f"w{k}")
            nc.sync.dma_start(wk[:], wT[k * P : (k + 1) * P, :])
            wt_lo.append(wk[:, 0:512])
            wt_hi.append(wk[:, 512:1024])

        xb = xpool.tile([P, KT * HB], in_dt, tag="xb")
        nc.sync.dma_start(
            xb[:].rearrange("p (k b) -> p k b", k=KT),
            xT[:, HB:BS].rearrange("(k p) b -> p k b", p=P),
        )

        def x_slice(k, bt):
            half, boff = divmod(bt * P, HB)
            if half:
                return xb[:, k * HB + boff : k * HB + boff + P]
            if k == 0:
                t = x0q0 if boff < 512 else x0q1
                return t[:, boff % 512 : boff % 512 + P]
            return xa[k][:, boff : boff + P]

        def finish(bt, lg, snb):
            dt_ = dpool.tile([P, C], out_dt)
            nc.scalar.activation(
                dt_[:],
                lg[:],
                mybir.ActivationFunctionType.Sqrt,
                bias=snb[:],
                scale=-2.0 * ALPHA,
            )
            nc.sync.dma_start(dist[bt * P : (bt + 1) * P, :], dt_[:])

        # The Sqrt + dist store for b-tile N are emitted after b-tile N+1's
        # copy/square chain: by then the bias operand (snb) has long been
        # produced, so ACT's in-order queue never idles waiting on the DVE
        # square/reduce chain.
        state = {"pending": None}

        def mm(bt, ps, k):
            lhs = x_slice(k, bt)
            nc.tensor.matmul(
                ps[:, 0:512], lhs, wt_lo[k], start=(k == 0), stop=(k == KT - 1)
            )
            nc.tensor.matmul(
                ps[:, 512:1024], lhs, wt_hi[k], start=(k == 0), stop=(k == KT - 1)
            )

        def epilogue(bt, ps):
            # ACT is the only PSUM consumer: one Copy pass materializes bf16
            # logits and releases the PSUM banks for the next b-tile.
            lg = lpool.tile([P, C], out_dt)
            nc.scalar.copy(lg[:], ps[:])

            sq = spool.tile([P, C], out_dt)
            nc.vector.tensor_tensor(sq[:], lg[:], lg[:], mybir.AluOpType.mult)
            sn = npool.tile([P, 1], F32, tag="sn")
            nc.vector.tensor_reduce(
                sn[:], sq[:], axis=mybir.AxisListType.X, op=mybir.AluOpType.add
            )
            snb = npool.tile([P, 1], F32, tag="snb")
            nc.vector.tensor_scalar_add(snb[:], sn[:], ALPHA * ALPHA)

            # logits only need the copy — store them a stage earlier than dist
            nc.sync.dma_start(logits[bt * P : (bt + 1) * P, :], lg[:])

            if state["pending"] is not None:
                finish(*state["pending"])
            state["pending"] = (bt, lg, snb)

        # b-tiles 0-7 run k-major in groups of 4: each (x_k, w_k) DMA piece
        # unlocks 8 matmuls across the group, so the PE stays busy (and HAM
        # stays warm) while the ramp-in loads stream. b-tiles 8-15 run per-
        # tile, which pipelines the epilogues one tile deep for a short tail.
        for g0 in (0, 4):
            pss = [
                psum.tile([P, C], F32, tag="ps", name=f"ps{g0 + i}") for i in range(4)
            ]
            for k in range(KT):
                for i in range(4):
                    mm(g0 + i, pss[i], k)
            for i in range(4):
                epilogue(g0 + i, pss[i])

        for bt in range(8, NBT - 1):
            ps = psum.tile([P, C], F32, tag="ps")
            for k in range(KT):
                mm(bt, ps, k)
            epilogue(bt, ps)

        # Last b-tile runs c-half-major: the lo half's copy/square/reduce
        # overlap the hi half's matmuls, shortening the post-matmul tail.
        bt = NBT - 1
        ps = psum.tile([P, C], F32, tag="ps", name="ps_last")
        lg = lpool.tile([P, C], out_dt, tag="lg", name="lg_last")
        sq = spool.tile([P, C], out_dt, tag="sq", name="sq_last")
        parts = []
        for c0, c1, wtch in ((0, 512, wt_lo), (512, 1024, wt_hi)):
            for k in range(KT):
                nc.tensor.matmul(
                    ps[:, c0:c1], x_slice(k, bt), wtch[k],
                    start=(k == 0), stop=(k == KT - 1),
                )
            nc.scalar.copy(lg[:, c0:c1], ps[:, c0:c1])
            nc.vector.tensor_tensor(
                sq[:, c0:c1], lg[:, c0:c1], lg[:, c0:c1], mybir.AluOpType.mult
            )
            pt = npool.tile([P, 1], F32, tag=f"pt{c0}", name=f"pt{c0}")
            nc.vector.tensor_reduce(
                pt[:], sq[:, c0:c1], axis=mybir.AxisListType.X,
                op=mybir.AluOpType.add,
            )
            parts.append(pt)
            if c0 == 0:
                finish(*state["pending"])  # sqrt(14) fills ACT during hi MMs
                state["pending"] = None
        snb = npool.tile([P, 1], F32, tag="snb", name="snb_last")
        nc.vector.tensor_tensor(
            snb[:], parts[0][:], parts[1][:], mybir.AluOpType.add
        )
        nc.vector.tensor_scalar_add(snb[:], snb[:], ALPHA * ALPHA)
        nc.sync.dma_start(logits[bt * P : (bt + 1) * P, :], lg[:])
        finish(bt, lg, snb)

    nc.compile()
    return nc


_NC = {}


def _round_tf32(a):
    """Round-to-nearest-even to TF32 (10-bit mantissa) in fp32 storage.

    The FP32r PE mode multiplies at TF32 precision and the BIR contract is
    that f32r operands arrive pre-rounded; carry into the exponent on
    mantissa overflow is exactly what RNE needs (inf/nan inputs don't occur
    here).
    """
    u = a.view(np.uint32)
    r = (u + np.uint32(0xFFF) + ((u >> np.uint32(13)) & np.uint32(1))) & np.uint32(
        0xFFFFE000
    )
    return r.view(np.float32)


def kernel(x, W, trace=False, _result_box=None, io_mode=IO_MODE):
    if io_mode not in _NC:
        _NC[io_mode] = build(io_mode)
    nc = _NC[io_mode]

    x = np.ascontiguousarray(np.asarray(x, dtype=np.float32))
    W = np.ascontiguousarray(np.asarray(W, dtype=np.float32))
    if io_mode == "bf16":
        prep = lambda a: np.asarray(a, dtype=ml_dtypes.bfloat16)
    else:
        prep = _round_tf32
    wT = prep(np.ascontiguousarray(W.T))
    in_maps = [
        {
            "xT": prep(np.ascontiguousarray(x[i * BS : (i + 1) * BS, :].T)),
            "wT": wT,
        }
        for i in range(N_CORES)
    ]

    # The first execution of a freshly loaded NEFF has been seen to flake
    # (transient NRT_EXEC_UNIT_UNRECOVERABLE / corrupt output on this
    # fabric); do a throwaway warm-up exec with one retry, then the real run.
    try:
        run_bass_kernel_spmd(nc, in_maps, list(range(N_CORES)))
    except Exception:
        try:
            run_bass_kernel_spmd(nc, in_maps, list(range(N_CORES)))
        except Exception:
            pass

    res = run_bass_kernel_spmd(nc, in_maps, list(range(N_CORES)), trace=trace)
    if _result_box is not None:
        _result_box.append(res)

    logits = np.concatenate(
        [np.asarray(res.results[i]["logits"], dtype=np.float32) for i in range(N_CORES)],
        axis=0,
    )
    dist = np.concatenate(
        [np.asarray(res.results[i]["dist"], dtype=np.float32) for i in range(N_CORES)],
        axis=0,
    )
    return logits, dist
